# revision 21
# baseline (speedup 1.0000x reference)
"""DigitCaps dynamic-routing kernel for 8 Trainium2 NeuronCores — v6.

Problem: x(32,16384,8) f32, W(10,16384,8,16) f32 -> v(32,10,16) f32
  u_hat[b,j,p,o] = sum_d x[b,p,d] W[j,p,d,o]   (never materialized)
  3 routing iterations (softmax over j, weighted sums over p).

Measured lineage: v1 596, v2 452, v3 565, v4 401, v5 451, v6 453us.
v6's trace: every DVE/gpsimd op passed 4D APs fell off its fast path
(mul 606->1638ns, gpsimd mul ->3030), the it=1 in-place f32 bb-add
cost 3440ns, it0 was dispatch-bound (128 mm pairs ~180ns each), and
the consolidated 1.28MB wz DMA serialized on ONE queue (~20us).

v8 = 356us measured (v7 380, baseline v4 401). KEY finding: gpsimd
shares the DVE SBUF port — ANY gpsimd compute slows concurrent DVE
ops up to 2.7x (identical flat-AP muls: 692 vs 1894ns). gpsimd is
used ONLY for DMA dispatch here.

v7/v8 (over v6's structure):
  * every elementwise op passes FLAT 2D/3D contiguous views — the
    (j,t)-major tmpJ/bb/e layouts make all of them flattenable
  * bb is bf16 and double-buffered: it=1 writes bbB = bbA + uv, no
    in-place f32 read-modify-write on the softmax critical path
  * it0 packs 2 d's per matmul (64 mm pairs, diag-pair drain fixed
    on V) and the cc0 drain spreads over sync/scalar/gpsimd queues
  * wz pull per (it,tg) split into 4 dma_starts on 4 queues
  * squash: reciprocal_approx_fast; v^T scattered into pre-zeroed
    [128,D,J,B] vblk; scatter/drain DMAs on sync+gpsimd queues
  * emission z0 z1 s0 z2 s1 z3 s2 s3; s j-groups (0,8),(8,2)
"""
import numpy as np
import ml_dtypes
from functools import lru_cache

import concourse.bacc as bacc
import concourse.mybir as mybir
from concourse import tile
from concourse.bass_utils import run_bass_kernel_spmd

F32 = mybir.dt.float32
BF16 = mybir.dt.bfloat16
AX = mybir.AxisListType
ALU = mybir.AluOpType
ACTF = mybir.ActivationFunctionType

B, J, P, D, O = 32, 10, 16384, 8, 16
NCORES = 8
PL = P // NCORES          # 2048
T = PL // 128             # 16 tiles of 128 p's
TG = 4                    # t-group size
NTG = T // TG             # 4
JO = J * O                # 160
JB = J * B                # 320
JT = J * TG               # 40
JGS = [(0, 8), (8, 2)]    # (j0, width) j-groups for s-phase

MUL_GPS = set()           # gpsimd compute steals DVE SBUF ports — keep it off


def _emit(nc, n_cores):
    xb = nc.dram_tensor("xb", [128, T, D, B], BF16, kind="ExternalInput")
    ws = nc.dram_tensor("ws", [128, T, D, J, O], BF16, kind="ExternalInput")
    wz = nc.dram_tensor("wz", [NTG, 128, J, TG, 128], BF16,
                        kind="ExternalInput")
    ones16 = nc.dram_tensor("ones16", [O, O], BF16, kind="ExternalInput")
    s3pT = nc.dram_tensor("s3pT", [O, J, B], F32, kind="ExternalOutput")

    dmaq = [nc.sync, nc.scalar, nc.gpsimd]

    with tile.TileContext(nc) as tc:
        with (
            tc.tile_pool(name="per", bufs=1) as per,
            tc.tile_pool(name="wu", bufs=1, space="DRAM") as wup,
            tc.tile_pool(name="ypool", bufs=7) as ypool,
            tc.tile_pool(name="sm", bufs=2) as sm,
            tc.tile_pool(name="tj", bufs=1) as tjp,
            tc.tile_pool(name="u1p", bufs=2) as u1p,
            tc.tile_pool(name="zscr", bufs=4) as zscrp,
            tc.tile_pool(name="wzg", bufs=2) as wzp,
            tc.tile_pool(name="small", bufs=1) as small,
            tc.tile_pool(name="sps", bufs=1, space="PSUM") as sps,
            tc.tile_pool(name="zps", bufs=2, space="PSUM") as zps,
            tc.tile_pool(name="dram", bufs=2, space="DRAM") as dramp,
        ):
            # warmup collective: junk contents on purpose — no input deps,
            # issues immediately, absorbs the first-collective barrier
            # (and most of the core-launch skew) under the it0 compute.
            wu_in = wup.tile([B, 16], F32)
            wu_out = wup.tile([B, 16], F32)
            nc.gpsimd.collective_compute(
                "AllReduce", ALU.add,
                replica_groups=[list(range(n_cores))],
                ins=[wu_in[:].opt()], outs=[wu_out[:].opt()],
            )

            x_sb = per.tile([128, T, D, B], BF16)
            nc.sync.dma_start(x_sb[:], xb[:, :, :, :])
            ws_t = []
            for t in range(T):
                w = per.tile([128, D, J, O], BF16, name=f"ws{t}")
                dmaq[t % 3].dma_start(w[:], ws[:, t, :, :, :])
                ws_t.append(w)
            one_sb = per.tile([O, O], BF16)
            nc.sync.dma_start(one_sb[:], ones16[:, :])
            # z-phase rhs: vblk[(d,o), d', j, b] = v[b,j,o] iff d==d'.
            # Off-diagonal slots zeroed once; every squash rewrites only
            # the D diagonal slots via strided DMA (640B runs).
            vblk = per.tile([128, D, J, B], BF16)
            nc.vector.memset(vblk[:], 0.0)
            # bb is t-group-major, then (j, t4): every per-tg consumer
            # (z-tree write, exp) sees one contiguous [128, J*TG*B] run.
            # bf16 + double-buffered: it=1 writes a fresh tile instead of
            # an in-place f32 accumulate (which measured 3.4us/tg).
            bbA = per.tile([128, NTG, J, TG, B], BF16)
            bbB = per.tile([128, NTG, J, TG, B], BF16)

            y_t = [None] * T

            def allreduce(src_dram):
                out = dramp.tile([O, J, B], F32)
                nc.gpsimd.collective_compute(
                    "AllReduce", ALU.add,
                    replica_groups=[list(range(n_cores))],
                    ins=[src_dram[:].opt()], outs=[out[:].opt()],
                )
                return out

            def squash_scatter(cc_out):
                """cc_out (DRAM [O,J,B] f32 summed s) -> vblk diagonal."""
                s_fT = small.tile([O, J, B], F32)
                nc.sync.dma_start(s_fT[:], cc_out[:, :, :])
                ssq = small.tile([O, JB], BF16)
                sfv = s_fT.rearrange("o j b -> o (j b)")
                nc.vector.tensor_mul(ssq[:], sfv, sfv)
                sq_ps = sps.tile([128, 512], F32, tag="s0ps", name="sq_ps")
                nc.tensor.matmul(sq_ps[0:O, 0:JB], one_sb[:], ssq[:],
                                 start=True, stop=True)
                sqv = small.tile([O, JB], F32)
                nc.vector.tensor_copy(sqv[:], sq_ps[0:O, 0:JB])
                r_ = small.tile([O, JB], F32)
                nc.scalar.activation(r_[:], sqv[:], ACTF.Sqrt)
                den = small.tile([O, JB], F32)
                nc.vector.scalar_tensor_tensor(
                    den[:], sqv[:], 1.0, r_[:], ALU.add, ALU.mult)
                rc = small.tile([O, JB], F32)
                nc.vector.reciprocal_approx_fast(rc[:], den[:])
                f_ = small.tile([O, JB], F32)
                nc.vector.tensor_mul(f_[:], sqv[:], rc[:])
                vT = small.tile([O, J, B], BF16)
                nc.vector.tensor_mul(
                    vT.rearrange("o j b -> o (j b)"), sfv, f_[:])
                for d in range(D):
                    dmaq[d % 3].dma_start(
                        vblk[d * O:(d + 1) * O, d, :, :], vT[:])

            # ---------------- it0 s-phase: c == 0.1 ----------------
            # 2 d's packed per matmul (64 dispatch pairs instead of 128);
            # the psum picks up garbage off-diagonal d-blocks — the drain
            # adds the two diagonal blocks and ignores the rest.
            s0_ps = sps.tile([128, 2, JO], F32, tag="s0ps")
            for t in range(T):
                for d2 in range(D // 2):
                    nc.tensor.matmul(
                        s0_ps[0:2 * B, :, :],
                        x_sb[:, t, 2 * d2:2 * d2 + 2, :],
                        ws_t[t][:, 2 * d2:2 * d2 + 2, :, :],
                        start=(t == 0 and d2 == 0),
                        stop=(t == T - 1 and d2 == D // 2 - 1),
                    )
            s_half = small.tile([B, JO], F32)
            nc.scalar.activation(s_half[:], s0_ps[0:B, 0, :], ACTF.Copy,
                                 scale=0.1)
            s_sb = small.tile([B, J, O], F32)
            nc.vector.scalar_tensor_tensor(
                s_sb.rearrange("b j o -> b (j o)"),
                s0_ps[B:2 * B, 1, :], 0.1, s_half[:],
                ALU.mult, ALU.add)
            cc0 = dramp.tile([O, J, B], F32)
            for j in range(J):
                dmaq[j % 3].dma_start(
                    cc0[:, j, :].rearrange("o b -> b o"), s_sb[:, j, :])
            squash_scatter(allreduce(cc0))

            def emit_softmax_y(bb, tg):
                """softmax_j(bb) for t-group tg; y(t) = e * (x*rec)."""
                t0 = tg * TG
                e_tg = sm.tile([128, J, TG, B], BF16)
                ef = e_tg.rearrange("p j t b -> p (j t b)")
                nc.scalar.activation(
                    ef, bb[:, tg].rearrange("p j t b -> p (j t b)"),
                    ACTF.Exp)
                # sum over j: contiguous halves tree, all flat 2D views
                TB = TG * B
                es1 = sm.tile([128, 5 * TB], BF16)
                nc.vector.tensor_add(es1[:], ef[:, 0:5 * TB],
                                     ef[:, 5 * TB:10 * TB])
                es2 = sm.tile([128, 2 * TB], BF16)
                nc.vector.tensor_add(es2[:], es1[:, 0:2 * TB],
                                     es1[:, 2 * TB:4 * TB])
                es3 = sm.tile([128, TB], BF16)
                nc.vector.tensor_add(es3[:], es2[:, 0:TB], es2[:, TB:2 * TB])
                se = sm.tile([128, TG, B], F32)
                nc.vector.tensor_add(
                    se.rearrange("p t b -> p (t b)"), es3[:],
                    es1[:, 4 * TB:5 * TB])
                rec = sm.tile([128, TG, B], BF16)
                with nc.allow_low_precision(
                        reason="1/sum feeds bf16 softmax muls"):
                    nc.vector.reciprocal(rec[:], se[:])
                # fold 1/sum into x once: y = e * xr stays on the DVE
                # double-broadcast fast path (~0.56ns/elem)
                xr = sm.tile([128, TG, D, B], BF16)
                nc.vector.tensor_mul(
                    xr[:], x_sb[:, t0:t0 + TG, :, :],
                    rec[:, :, None, :].broadcast_to([128, TG, D, B]))
                for t4 in range(TG):
                    y = ypool.tile([128, J, D, B], BF16)
                    nc.vector.tensor_mul(
                        y[:],
                        e_tg[:, :, t4, None, :].broadcast_to([128, J, D, B]),
                        xr[:, t4, None, :, :].broadcast_to([128, J, D, B]))
                    y_t[t0 + t4] = y

            def emit_z_tg(it, tg):
                """z matmuls + x-weighted d-sum for t-group tg -> bb."""
                wzg = wzp.tile([128, J, TG, 128], BF16)
                if it == 0 and tg <= 1:
                    # fake dep: delay the wz pull until the it0 drain
                    # exists so x+ws own the DMA queues at t=0
                    nc.gpsimd.tensor_copy(wzg[0:1, 0, 0, 0:2],
                                          s_sb[0:1, 0, 0:2])
                # 4 dma_starts so the pull spreads over 4 HW queues
                for j0, j1 in ((0, 3), (3, 6), (6, 8), (8, 10)):
                    nc.sync.dma_start(wzg[:, j0:j1], wz[tg, :, j0:j1, :, :])
                tmpJ = tjp.tile([128, J, TG, D, B], BF16)
                xvf = (x_sb[:, tg * TG:(tg + 1) * TG, :, :]
                       .rearrange("p t d b -> p (t d b)"))
                for j in range(J):
                    z_ps = zps.tile([128, TG, 256], F32)
                    for t4 in range(TG):
                        nc.tensor.matmul(
                            z_ps[:, t4, :],
                            wzg[:, j, t4, :],
                            vblk[:, :, j, :],
                            start=(t4 % 2 == 0), stop=(t4 % 2 == 1))
                    zscr = zscrp.tile([128, TG * D * B], BF16)
                    # at the iteration-start ramp (tg 0) V is idle while S
                    # serially drains psum — let V take 3 of the 10 copies
                    ceng = nc.vector if (tg == 0 and j < 3) else nc.scalar
                    if ceng is nc.vector:
                        nc.vector.tensor_copy(
                            zscr[:], z_ps.rearrange("p t db -> p (t db)"))
                    else:
                        nc.scalar.copy(
                            zscr[:], z_ps.rearrange("p t db -> p (t db)"))
                    eng = nc.gpsimd if j in MUL_GPS else nc.vector
                    eng.tensor_mul(
                        tmpJ[:, j].rearrange("p t d b -> p (t d b)"),
                        zscr[:], xvf)
                # batched d-add-tree over all (j,t), flat 3D views
                tjf = tmpJ.rearrange("p j t d b -> p (j t) (d b)")
                u1 = u1p.tile([128, JT, 4 * B], BF16)
                nc.vector.tensor_add(u1[:], tjf[:, :, 0:4 * B],
                                     tjf[:, :, 4 * B:8 * B])
                nc.vector.tensor_add(u1[:, :, 0:2 * B], u1[:, :, 0:2 * B],
                                     u1[:, :, 2 * B:4 * B])
                bb = bbA if it == 0 else bbB
                bb_v = bb[:, tg].rearrange("p j t b -> p (j t) b")
                if it == 0:
                    nc.vector.tensor_add(bb_v, u1[:, :, 0:B],
                                         u1[:, :, B:2 * B])
                else:
                    uv3 = u1p.tile([128, JT, B], BF16, name="uv3")
                    nc.vector.tensor_add(uv3[:], u1[:, :, 0:B],
                                         u1[:, :, B:2 * B])
                    nc.vector.tensor_add(
                        bb_v, bbA[:, tg].rearrange("p j t b -> p (j t) b"),
                        uv3[:])
                emit_softmax_y(bb, tg)

            def emit_s_chunk(ps_jg, chunk):
                """s matmuls for 4 t's, j-group-contiguous (LDW pipelines).
                stationary = ws [128,(jw,o)], moving = y [128,(jw),(b)],
                out[(j,o),(j,b)] accumulated over all (t,d)."""
                for gi, (j0, jw) in enumerate(JGS):
                    for t in range(chunk * TG, (chunk + 1) * TG):
                        for d in range(D):
                            nc.tensor.matmul(
                                ps_jg[gi][0:jw * O, 0:jw * B],
                                ws_t[t][:, d, j0:j0 + jw, :],
                                y_t[t][:, j0:j0 + jw, d, :],
                                start=(t == 0 and d == 0),
                                stop=(t == T - 1 and d == D - 1),
                            )

            def drain_s(ps_jg, dst):
                """psum diag blocks [(j,o),(j,b)] -> dst[o, j, b] DRAM."""
                for gi, (j0, jw) in enumerate(JGS):
                    zsb = small.tile([128, 256], F32, name=f"zsb{gi}")
                    nc.vector.tensor_copy(
                        zsb[0:jw * O, 0:jw * B],
                        ps_jg[gi][0:jw * O, 0:jw * B])
                    for jl in range(jw):
                        dmaq[2 * ((j0 + jl) % 2)].dma_start(
                            dst[:, j0 + jl, :],
                            zsb[jl * O:(jl + 1) * O, jl * B:(jl + 1) * B])

            # ---------------- routing iterations ----------------
            for it in range(2):
                last = (it == 1)
                ps_jg = [sps.tile([128, jw * B], F32, name=f"spsj{gi}")
                         for gi, (j0, jw) in enumerate(JGS)]
                emit_z_tg(it, 0)
                emit_z_tg(it, 1)
                emit_s_chunk(ps_jg, 0)
                emit_z_tg(it, 2)
                emit_s_chunk(ps_jg, 1)
                emit_z_tg(it, 3)
                emit_s_chunk(ps_jg, 2)
                emit_s_chunk(ps_jg, 3)
                if last:
                    drain_s(ps_jg, s3pT)
                else:
                    cc_in = dramp.tile([O, J, B], F32)
                    drain_s(ps_jg, cc_in)
                    squash_scatter(allreduce(cc_in))
    return nc


@lru_cache(maxsize=2)
def _build(n_cores):
    nc = bacc.Bacc("TRN2", target_bir_lowering=False, debug=False,
                   num_devices=n_cores)
    _emit(nc, n_cores)
    nc.compile()
    return nc


def _prep_inputs(x, W):
    """Host-side shard + relayout. Returns list of per-core input dicts."""
    x = np.asarray(x, dtype=np.float32)
    W = np.asarray(W, dtype=np.float32)
    one = np.ones((O, O), np.float32).astype(ml_dtypes.bfloat16)
    in_maps = []
    for c in range(NCORES):
        xc = x[:, c * PL:(c + 1) * PL, :]              # (B, PL, D)
        Wc = W[:, c * PL:(c + 1) * PL, :, :]           # (J, PL, D, O)
        xr = np.ascontiguousarray(
            xc.reshape(B, T, 128, D).transpose(2, 1, 3, 0))        # [128,T,D,B]
        wsr = np.ascontiguousarray(
            Wc.reshape(J, T, 128, D, O).transpose(2, 1, 3, 0, 4))  # [128,T,D,J,O]
        wzr = np.ascontiguousarray(
            Wc.reshape(J, T, 128, D, O).transpose(0, 3, 4, 1, 2)   # j,d,o,t,p
            .reshape(J, 128, NTG, TG, 128)                         # j,(d,o),tg,t4,p
            .transpose(2, 1, 0, 3, 4))                             # [NTG,128,J,TG,128]
        in_maps.append({
            "xb": xr.astype(ml_dtypes.bfloat16),
            "ws": wsr.astype(ml_dtypes.bfloat16),
            "wz": wzr.astype(ml_dtypes.bfloat16),
            "ones16": one,
        })
    return in_maps


def _squash_np(s):
    sq = np.sum(s * s, axis=-1, keepdims=True)
    return s * (sq / ((1.0 + sq) * np.sqrt(sq)))


def kernel(x, W):
    nc = _build(NCORES)
    in_maps = _prep_inputs(x, W)
    res = run_bass_kernel_spmd(nc, in_maps, list(range(NCORES)))
    s3 = np.zeros((B, J, O), np.float64)
    for r in res.results:
        s3 += r["s3pT"].astype(np.float64).transpose(2, 1, 0)
    v = _squash_np(s3)
    return v.astype(np.float32)
